# revision 6
# baseline (speedup 1.0000x reference)
"""GRU cell on 8 Trainium2 NeuronCores.

Reference computation (B=65536, D=256):
    z = sigmoid(x@Wz + h@Uz + bz)
    r = sigmoid(x@Wr + h@Ur + br)
    h_hat = tanh(x@Wh + (r*h)@Uh + bh)
    h_t = z*h + (1-z)*h_hat  ; returns (h_t, h_t)

Strategy: data-parallel over the batch dim (8 shards of 8192 rows).
The host pre-transposes each shard to [256, B_shard] so every on-chip
tensor lives in [hidden, batch] layout: the contraction dim of all six
GEMMs is then the SBUF partition dim with no on-chip transposes at all,
biases become per-partition ACT bias vectors, and the elementwise gate
math runs in the same layout the matmuls produce.  Matmul operands are
bitcast to float32r (full-rate PE mode for fp32 data).
"""

import os
import sys

for _p in ("/opt/trn_rl_repo", "/root/.axon_site/_ro/trn_rl_repo"):
    if os.path.isdir(_p) and _p not in sys.path:
        sys.path.append(_p)

import numpy as np

B = 65536
D = 256
N_CORES = 8
S = B // N_CORES  # batch rows per core
CH = 512  # batch columns per chunk (one PSUM bank of fp32)

_WNAMES = ("Wz", "Uz", "Wr", "Ur", "Wh", "Uh")
_BNAMES = ("bz", "br", "bh")


def build_nc(s=S, mm_dtype_name=None, ch=CH):
    """Build + compile the per-core Bass program for a shard of s rows."""
    import concourse.bass as bass
    import concourse.mybir as mybir
    import concourse.tile as tile
    from concourse import bacc

    f32 = mybir.dt.float32
    if mm_dtype_name is None:
        mm_dtype_name = os.environ.get("GRU_MM_DTYPE", "float32r")
    mm_dt = getattr(mybir.dt, mm_dtype_name)
    AF = mybir.ActivationFunctionType

    nc = bacc.Bacc("TRN2", target_bir_lowering=False)
    xT = nc.dram_tensor("xT", [D, s], f32, kind="ExternalInput")
    hT = nc.dram_tensor("hT", [D, s], f32, kind="ExternalInput")
    w_d = {n: nc.dram_tensor(n, [D, D], f32, kind="ExternalInput") for n in _WNAMES}
    b_d = {n: nc.dram_tensor(n, [D], f32, kind="ExternalInput") for n in _BNAMES}
    outT = nc.dram_tensor("outT", [D, s], f32, kind="ExternalOutput")

    nch = s // ch
    cast = mm_dt != f32
    # float32r is bit-identical to float32; allocate matmul operand tiles as
    # f32r and bitcast the fp32 views where engines need plain f32 semantics.
    f32r_mode = mm_dt == mybir.dt.float32r

    def md(ap):
        if ap.dtype == mm_dt:
            return ap
        return ap.bitcast(mm_dt) if cast else ap

    with tile.TileContext(nc) as tc:
        with (
            tc.tile_pool(name="const", bufs=1) as cpool,
            tc.tile_pool(name="inp", bufs=3) as ipool,
            tc.tile_pool(name="work", bufs=3) as wpool,
            tc.tile_pool(name="psum", bufs=1, space=bass.MemorySpace.PSUM) as ppool,
        ):
            # --- constants: weights [128, 256] x2 k-chunks each, biases [128, 2]
            w_sb = {}
            for n in _WNAMES:
                for k in range(2):
                    src = w_d[n][k * 128 : (k + 1) * 128, :]
                    if f32r_mode:
                        t = cpool.tile([128, D], mm_dt, tag=f"w_{n}_{k}")
                        nc.sync.dma_start(t[:], src.bitcast(mm_dt))
                    elif cast:
                        t0 = cpool.tile([128, D], f32, tag=f"wld_{n}_{k}")
                        nc.sync.dma_start(t0[:], src)
                        t = cpool.tile([128, D], mm_dt, tag=f"w_{n}_{k}")
                        nc.vector.tensor_copy(t[:], t0[:])
                    else:
                        t = cpool.tile([128, D], f32, tag=f"w_{n}_{k}")
                        nc.sync.dma_start(t[:], src)
                    w_sb[(n, k)] = t
            b_sb = {}
            for n in _BNAMES:
                t = cpool.tile([128, 2], f32, tag=f"b_{n}")
                nc.sync.dma_start(t[:], b_d[n].rearrange("(g p) -> p g", p=128))
                b_sb[n] = t

            def gate_psum(pool_tag, wn, un, rhs_w, rhs_u, g):
                """psum[{128},{ch}] = W[:,g].T @ rhs_w + U[:,g].T @ rhs_u."""
                p = ppool.tile([128, ch], f32, tag=pool_tag)
                gs = slice(g * 128, (g + 1) * 128)
                nc.tensor.matmul(p[:], md(w_sb[(wn, 0)][:, gs]), md(rhs_w[0][:]),
                                 start=True, stop=False)
                nc.tensor.matmul(p[:], md(w_sb[(wn, 1)][:, gs]), md(rhs_w[1][:]),
                                 start=False, stop=False)
                nc.tensor.matmul(p[:], md(w_sb[(un, 0)][:, gs]), md(rhs_u[0][:]),
                                 start=False, stop=False)
                nc.tensor.matmul(p[:], md(w_sb[(un, 1)][:, gs]), md(rhs_u[1][:]),
                                 start=False, stop=True)
                return p

            for c in range(nch):
                cols = slice(c * ch, (c + 1) * ch)
                # xt/ht: matmul-operand tiles; htf: f32 views of h for the
                # elementwise gate math.
                xt, ht, htf = [], [], []
                for k in range(2):
                    if f32r_mode:
                        tx = ipool.tile([128, ch], mm_dt, tag=f"x{k}")
                        nc.sync.dma_start(
                            tx[:], xT[k * 128 : (k + 1) * 128, cols].bitcast(mm_dt)
                        )
                        th = ipool.tile([128, ch], mm_dt, tag=f"h{k}")
                        nc.sync.dma_start(
                            th[:], hT[k * 128 : (k + 1) * 128, cols].bitcast(mm_dt)
                        )
                        xt.append(tx)
                        ht.append(th)
                        htf.append(th[:].bitcast(f32))
                    else:
                        tx = ipool.tile([128, ch], f32, tag=f"x{k}")
                        nc.sync.dma_start(tx[:], xT[k * 128 : (k + 1) * 128, cols])
                        th = ipool.tile([128, ch], f32, tag=f"h{k}")
                        nc.sync.dma_start(th[:], hT[k * 128 : (k + 1) * 128, cols])
                        htf.append(th[:])
                        if cast:
                            cx = ipool.tile([128, ch], mm_dt, tag=f"xc{k}")
                            nc.vector.tensor_copy(cx[:], tx[:])
                            chh = ipool.tile([128, ch], mm_dt, tag=f"hc{k}")
                            nc.vector.tensor_copy(chh[:], th[:])
                            xt.append(cx)
                            ht.append(chh)
                        else:
                            xt.append(tx)
                            ht.append(th)

                # reset gate -> r*h (needed before the candidate matmuls)
                rh = []
                for g in range(2):
                    pr = gate_psum(f"pr{g}", "Wr", "Ur", xt, ht, g)
                    rt = wpool.tile([128, ch], f32, tag=f"r{g}")
                    nc.scalar.activation(rt[:], pr[:], AF.Sigmoid,
                                         bias=b_sb["br"][:, g : g + 1])
                    t = wpool.tile([128, ch], mm_dt if cast else f32, tag=f"rh{g}")
                    nc.vector.tensor_mul(t[:], rt[:], htf[g])
                    rh.append(t)

                # update gate
                zt = []
                for g in range(2):
                    pz = gate_psum(f"pz{g}", "Wz", "Uz", xt, ht, g)
                    t = wpool.tile([128, ch], f32, tag=f"z{g}")
                    nc.scalar.activation(t[:], pz[:], AF.Sigmoid,
                                         bias=b_sb["bz"][:, g : g + 1])
                    zt.append(t)

                # candidate + combine + store
                for g in range(2):
                    ph = gate_psum(f"ph{g}", "Wh", "Uh", xt, rh, g)
                    hh = wpool.tile([128, ch], f32, tag=f"hh{g}")
                    nc.scalar.activation(hh[:], ph[:], AF.Tanh,
                                         bias=b_sb["bh"][:, g : g + 1])
                    d = wpool.tile([128, ch], f32, tag=f"d{g}")
                    nc.vector.tensor_sub(d[:], htf[g], hh[:])
                    m = wpool.tile([128, ch], f32, tag=f"m{g}")
                    nc.vector.tensor_mul(m[:], zt[g][:], d[:])
                    o = wpool.tile([128, ch], f32, tag=f"o{g}")
                    nc.vector.tensor_add(o[:], hh[:], m[:])
                    nc.sync.dma_start(outT[g * 128 : (g + 1) * 128, cols], o[:])

    nc.compile()
    return nc


_NC_CACHE = {}


def _get_nc():
    key = (S, os.environ.get("GRU_MM_DTYPE", "float32r"), CH)
    if key not in _NC_CACHE:
        _NC_CACHE[key] = build_nc(S, key[1], CH)
    return _NC_CACHE[key]


def _make_in_maps(inputs):
    f32 = np.float32
    x = np.asarray(inputs["x"], f32)
    h = np.asarray(inputs["h_t_1"], f32)
    consts = {n: np.ascontiguousarray(np.asarray(inputs[n], f32)) for n in _WNAMES}
    consts.update(
        {n: np.ascontiguousarray(np.asarray(inputs[n], f32)) for n in _BNAMES}
    )
    in_maps = []
    for c in range(N_CORES):
        sl = slice(c * S, (c + 1) * S)
        m = {
            "xT": np.ascontiguousarray(x[sl].T),
            "hT": np.ascontiguousarray(h[sl].T),
        }
        m.update(consts)
        in_maps.append(m)
    return in_maps


def run(inputs, trace=False):
    """Run on hardware; returns (h_t ndarray, BassKernelResults)."""
    from concourse.bass_utils import run_bass_kernel_spmd

    nc = _get_nc()
    in_maps = _make_in_maps(inputs)
    res = run_bass_kernel_spmd(nc, in_maps, list(range(N_CORES)), trace=trace)
    out = np.empty((B, D), np.float32)
    for c in range(N_CORES):
        out[c * S : (c + 1) * S] = res.results[c]["outT"].T
    return out, res


def kernel(**inputs):
    out, _ = run(inputs, trace=False)
    return (out, out)
